# revision 1
# baseline (speedup 1.0000x reference)
"""Class-wise whitening-coloring transform (CWCT) on 8 Trainium2 NeuronCores.

Strategy (per the sharding hint, pixels are sharded across devices):
 * Host sorts pixels by segmentation label (stable argsort of the int32 seg
   maps), splits each label's pixel run evenly across the 8 cores, and pads
   each per-core per-label run to a multiple of 128 pixels with zeros.
 * Phase A (device): per-label second moments S_l = sum x x^T as [256,256]
   matmul accumulation over 128-pixel tiles (pixel-on-partition layout,
   contraction along partitions).  Per-core partials are summed on the host
   (the [C,C] "all-reduce" of the hint).
 * Host: per-label means/covariances, guide gating, float64 Cholesky of the
   tiny 256x256 matrices (replicated work per the hint), builds T_l and bias.
 * Phase B (device): per-pixel color transform y = T_l x + c_l over each
   label's pixel run (channel-on-partition layout), embarrassingly parallel
   over pixels.
 * Host scatters transformed pixels back into the full [1,256,512,512] image.

Sorting pixels by label means every pixel enters exactly one covariance
matmul and one transform matmul (8x fewer FLOPs than masked per-label
matmuls), which is what puts this kernel at the memory/compute ridge.
Inputs are fed to the device as float16 (10-bit mantissa): the covariance
error this induces is ~1e-5 per entry and the end-to-end output error is
~3e-4 relative, while halving DMA traffic and running the PE at full rate.

Phase A inputs are laid out partition-major on the host (the exact SBUF
image, [128, ntiles*256]) so every DMA is a single dense copy with
multi-KB per-partition runs.
"""
import os
import sys

for _p in ("/opt/trn_rl_repo", "/root/.axon_site/_ro/trn_rl_repo"):
    if os.path.isdir(_p) and _p not in sys.path:
        sys.path.insert(0, _p)

# The bass kernels execute through jax's axon platform; make sure it is
# available even if the calling process pinned JAX_PLATFORMS=cpu.
if "jax" not in sys.modules:
    _plat = os.environ.get("JAX_PLATFORMS", "")
    if _plat and "axon" not in _plat:
        os.environ["JAX_PLATFORMS"] = "axon," + _plat
    elif not _plat:
        os.environ["JAX_PLATFORMS"] = "axon,cpu"

import numpy as np

import concourse.bass as bass
import concourse.tile as tile
from concourse import bacc, mybir

N_CORES = 8
NUM_LABELS = 8
C = 256
P = 128
HALF = 2  # channel halves (256 = 2*128)

DT_A = mybir.dt.float16       # phase A matmul/input dtype
NP_A = np.float16
DT_B_IN = mybir.dt.float16    # phase B x/T dtype
NP_B_IN = np.float16
DT_B_OUT = mybir.dt.float16   # phase B output dtype
NP_B_OUT = np.float16

PXCHUNK_B = 1024              # phase B pixels per DMA chunk (mult of 512)
MAX_TILES_CHUNK_A = 96        # phase A max 128-px tiles per DMA

_prog_cache = {}


def _new_nc():
    return bacc.Bacc("TRN2", target_bir_lowering=False, debug=False,
                     num_devices=N_CORES)


def build_phase_a(tiles_c, tiles_s, repeat=1, no_mm=False):
    """tiles_c/tiles_s: per processed label, CAP/128 tile counts.

    Inputs are partition-major: [128, sum(tiles)*256] where free offset
    t*256+c holds pixel (t*128 + partition), channel c.
    """
    nL = len(tiles_c)
    nc = _new_nc()
    xc = nc.dram_tensor("xc", [P, max(sum(tiles_c), 1) * C], DT_A,
                        kind="ExternalInput")
    xs = nc.dram_tensor("xs", [P, max(sum(tiles_s), 1) * C], DT_A,
                        kind="ExternalInput")
    mom = nc.dram_tensor("mom", [2, NUM_LABELS, HALF, P, C], mybir.dt.float32,
                         kind="ExternalOutput")
    tiles_per = [tiles_c, tiles_s]
    with tile.TileContext(nc) as tc:
        with (
            tc.tile_pool(name="in", bufs=3) as pin,
            tc.tile_pool(name="ps", bufs=2, space="PSUM") as pps,
            tc.tile_pool(name="so", bufs=3) as pout,
        ):
            def body_a(_=None):
                for i, src in enumerate([xc, xs]):
                    toff = 0
                    for li in range(nL):
                        ntile = tiles_per[i][li]
                        if ntile == 0:
                            continue
                        ps0 = pps.tile([P, C], mybir.dt.float32)
                        ps1 = pps.tile([P, C], mybir.dt.float32)
                        done = 0
                        while done < ntile:
                            cur = min(ntile - done, MAX_TILES_CHUNK_A)
                            xt = pin.tile([P, MAX_TILES_CHUNK_A * C], DT_A,
                                          tag="achunk")
                            nc.sync.dma_start(
                                xt[:, 0:cur * C],
                                src[:, (toff + done) * C:
                                    (toff + done + cur) * C])
                            for t in range(cur):
                                if no_mm:
                                    continue
                                xv = xt[:, bass.ts(t, C)]
                                st = (done + t == 0)
                                sp = (done + t == ntile - 1)
                                nc.tensor.matmul(ps0[:], xv[:, 0:P], xv[:],
                                                 start=st, stop=sp)
                                nc.tensor.matmul(ps1[:], xv[:, P:C], xv[:],
                                                 start=st, stop=sp)
                            done += cur
                        so0 = pout.tile([P, C], mybir.dt.float32)
                        so1 = pout.tile([P, C], mybir.dt.float32)
                        if no_mm:
                            nc.vector.tensor_copy(so0[:], xt[:, 0:C])
                            nc.vector.tensor_copy(so1[:], xt[:, 0:C])
                        else:
                            nc.vector.tensor_copy(so0[:], ps0[:])
                            nc.vector.tensor_copy(so1[:], ps1[:])
                        nc.scalar.dma_start(mom[i, li, 0], so0[:])
                        nc.scalar.dma_start(mom[i, li, 1], so1[:])
                        toff += ntile
            if repeat == 1:
                body_a()
            else:
                with tc.For_i(0, repeat, 1):
                    body_a()
    nc.compile()
    return nc


def build_phase_b(caps, repeat=1, bufs=4, chunk=None):
    """caps: per processed label, pixel capacity (multiple of 128)."""
    chunk = chunk or PXCHUNK_B
    nL = len(caps)
    ppad = sum(caps)
    nc = _new_nc()
    x = nc.dram_tensor("x", [HALF * P, ppad], DT_B_IN, kind="ExternalInput")
    tmat = nc.dram_tensor("tmat", [nL, HALF, HALF, P, P], DT_B_IN,
                          kind="ExternalInput")
    bvec = nc.dram_tensor("bvec", [HALF * P, nL], mybir.dt.float32,
                          kind="ExternalInput")
    y = nc.dram_tensor("y", [HALF * P, ppad], DT_B_OUT, kind="ExternalOutput")
    xv = x.rearrange("(h p) n -> p h n", h=HALF)
    yv = y.rearrange("(h p) n -> p h n", h=HALF)
    bv = bvec.rearrange("(h p) l -> p h l", h=HALF)
    with tile.TileContext(nc) as tc:
        with (
            tc.tile_pool(name="in", bufs=bufs) as pin,
            tc.tile_pool(name="tm", bufs=2) as ptm,
            tc.tile_pool(name="bias", bufs=1) as pb,
            tc.tile_pool(name="ps", bufs=4, space="PSUM") as pps,
            tc.tile_pool(name="out", bufs=bufs) as pout,
        ):
            bias = pb.tile([P, HALF * nL], mybir.dt.float32)
            nc.sync.dma_start(
                bias[:].rearrange("p (h l) -> p h l", h=HALF), bv[:])

            def body_b(_=None):
                base = 0
                for li in range(nL):
                    cap = caps[li]
                    tm = ptm.tile([P, 4 * P], DT_B_IN)
                    nc.sync.dma_start(
                        tm[:].rearrange("p (g q) -> p g q", q=P),
                        tmat[li].rearrange("a b p q -> p (a b) q"))
                    done = 0
                    while done < cap:
                        pxc = min(cap - done, chunk)
                        xt = pin.tile([P, HALF * chunk], DT_B_IN,
                                      tag="bchunk")
                        nc.sync.dma_start(
                            xt[:, 0:HALF * pxc].rearrange(
                                "p (h n) -> p h n", h=HALF),
                            xv[:, :, base + done:base + done + pxc])
                        yt = pout.tile([P, HALF * chunk], DT_B_OUT,
                                       tag="bout")
                        m0 = 0
                        while m0 < pxc:
                            mw = min(pxc - m0, 512)
                            for co in range(HALF):
                                ps = pps.tile([P, 512], mybir.dt.float32)
                                for ci in range(HALF):
                                    nc.tensor.matmul(
                                        ps[:, 0:mw],
                                        tm[:, bass.ts(ci * HALF + co, P)],
                                        xt[:, ci * pxc + m0:
                                           ci * pxc + m0 + mw],
                                        start=(ci == 0), stop=(ci == 1))
                                nc.scalar.activation(
                                    yt[:, co * pxc + m0:co * pxc + m0 + mw],
                                    ps[:, 0:mw],
                                    mybir.ActivationFunctionType.Identity,
                                    bias=bias[:, co * nL + li:
                                              co * nL + li + 1])
                            m0 += mw
                        nc.scalar.dma_start(
                            yv[:, :, base + done:base + done + pxc],
                            yt[:, 0:HALF * pxc].rearrange(
                                "p (h n) -> p h n", h=HALF))
                        done += pxc
                    base += cap
            if repeat == 1:
                body_b()
            else:
                with tc.For_i(0, repeat, 1):
                    body_b()
    nc.compile()
    return nc


def _axon_devices():
    import jax
    try:
        devs = jax.devices("axon")
    except Exception:
        devs = jax.devices()
    assert len(devs) >= N_CORES, f"need {N_CORES} neuron cores, have {devs}"
    return devs[:N_CORES]


def _run_spmd(nc, in_maps):
    """SPMD execute `nc` on the 8 axon-tunneled NeuronCores.

    Same mechanics as concourse.bass2jax.run_bass_via_pjrt, but pins the
    axon platform explicitly so it works no matter what JAX_PLATFORMS the
    calling process uses.
    """
    import jax
    from jax.sharding import Mesh, PartitionSpec
    from jax.experimental.shard_map import shard_map
    from concourse.bass2jax import (_bass_exec_p, install_neuronx_cc_hook,
                                    partition_id_tensor)

    install_neuronx_cc_hook()
    partition_name = (nc.partition_id_tensor.name
                      if nc.partition_id_tensor else None)
    in_names, out_names, out_avals, zero_outs = [], [], [], []
    for alloc in nc.m.functions[0].allocations:
        if not isinstance(alloc, mybir.MemoryLocationSet):
            continue
        name = alloc.memorylocations[0].name
        if alloc.kind == "ExternalInput":
            if name != partition_name:
                in_names.append(name)
        elif alloc.kind == "ExternalOutput":
            shape = tuple(alloc.tensor_shape)
            dtype = mybir.dt.np(alloc.dtype)
            out_names.append(name)
            out_avals.append(jax.core.ShapedArray(shape, dtype))
            zero_outs.append(np.zeros(shape, dtype))
    n_params = len(in_names)
    all_in_names = list(in_names) + list(out_names)
    if partition_name is not None:
        all_in_names.append(partition_name)

    def _body(*args):
        operands = list(args)
        if partition_name is not None:
            operands.append(partition_id_tensor())
        outs = _bass_exec_p.bind(
            *operands,
            out_avals=tuple(out_avals),
            in_names=tuple(all_in_names),
            out_names=tuple(out_names),
            lowering_input_output_aliases=(),
            sim_require_finite=True,
            sim_require_nnan=True,
            nc=nc,
        )
        return tuple(outs)

    mesh = Mesh(np.asarray(_axon_devices()), ("core",))
    in_specs = (PartitionSpec("core"),) * (n_params + len(out_names))
    out_specs = (PartitionSpec("core"),) * len(out_names)
    fn = jax.jit(
        shard_map(_body, mesh=mesh, in_specs=in_specs, out_specs=out_specs,
                  check_rep=False),
        keep_unused=True,
    )
    concat_in = [
        np.concatenate([np.asarray(in_maps[c][n]) for c in range(N_CORES)], 0)
        for n in in_names
    ]
    concat_zero = [
        np.zeros((N_CORES * z.shape[0], *z.shape[1:]), z.dtype)
        for z in zero_outs
    ]
    outs = fn(*concat_in, *concat_zero)
    res = []
    for c in range(N_CORES):
        d = {}
        for i, name in enumerate(out_names):
            a = np.asarray(outs[i]).reshape(N_CORES, *out_avals[i].shape)
            d[name] = a[c]
        res.append(d)
    return res


def _split_sizes(count, parts):
    q, r = divmod(count, parts)
    return [q + (1 if k < r else 0) for k in range(parts)]


def _prepare(lab, guide_labels):
    """Sort pixel indices by label, split per core.

    Returns: segs[k][li] = index array for core k, processed-label li;
             caps[li] = padded per-core capacity (multiple of 128).
    """
    order = np.argsort(lab, kind="stable")
    counts = np.bincount(lab, minlength=NUM_LABELS)
    starts = np.concatenate([[0], np.cumsum(counts)[:-1]])
    segs = [[] for _ in range(N_CORES)]
    caps = []
    for l in guide_labels:
        cnt = int(counts[l])
        sizes = _split_sizes(cnt, N_CORES)
        cap = max((max(sizes) + P - 1) // P * P, P)
        caps.append(cap)
        off = int(starts[l])
        for k in range(N_CORES):
            segs[k].append(order[off:off + sizes[k]])
            off += sizes[k]
    return segs, caps, counts


def kernel(content_feat, style_feat, content_seg, style_seg):
    content_feat = np.asarray(content_feat)
    style_feat = np.asarray(style_feat)
    content_seg = np.asarray(content_seg)
    style_seg = np.asarray(style_seg)

    B, Cc, H, W = content_feat.shape
    N = H * W
    x = content_feat.reshape(Cc, N)
    s = style_feat.reshape(Cc, N)
    labc = content_seg.reshape(-1)
    labs = style_seg.reshape(-1)

    counts_c = np.bincount(labc, minlength=NUM_LABELS).astype(np.float64)
    counts_s = np.bincount(labs, minlength=NUM_LABELS).astype(np.float64)
    guide = [(counts_c[l] > 10) and (counts_s[l] > 10)
             and (counts_c[l] < 100.0 * counts_s[l])
             and (counts_s[l] < 100.0 * counts_c[l])
             for l in range(NUM_LABELS)]
    glabels = [l for l in range(NUM_LABELS) if guide[l]]
    out = content_feat.astype(np.float32, copy=True)
    if not glabels:
        return out

    segs_c, caps_c, _ = _prepare(labc, glabels)
    segs_s, caps_s, _ = _prepare(labs, glabels)

    # transposed fp16 copies for gathering pixel rows
    xt16 = np.ascontiguousarray(x.T).astype(NP_A)   # [N, C]
    st16 = np.ascontiguousarray(s.T).astype(NP_A)

    ppad_c = sum(caps_c)
    ppad_s = sum(caps_s)
    offs_c = np.concatenate([[0], np.cumsum(caps_c)]).astype(int)
    offs_s = np.concatenate([[0], np.cumsum(caps_s)]).astype(int)

    # pixel-major gathered arrays [ppad, C]
    XT_c = np.zeros((N_CORES, ppad_c, C), NP_A)
    XT_s = np.zeros((N_CORES, ppad_s, C), NP_A)
    for k in range(N_CORES):
        for li in range(len(glabels)):
            seg = segs_c[k][li]
            XT_c[k, offs_c[li]:offs_c[li] + len(seg)] = xt16[seg]
            seg = segs_s[k][li]
            XT_s[k, offs_s[li]:offs_s[li] + len(seg)] = st16[seg]

    # partition-major images for phase A: [128, ntiles*256]
    def to_pm(a):  # [ppad, C] -> [P, (ppad//P)*C]
        t = a.reshape(-1, P, C).transpose(1, 0, 2)
        return np.ascontiguousarray(t).reshape(P, -1)

    tiles_c = [cap // P for cap in caps_c]
    tiles_s = [cap // P for cap in caps_s]
    key = ("A", tuple(tiles_c), tuple(tiles_s))
    if key not in _prog_cache:
        _prog_cache[key] = build_phase_a(tiles_c, tiles_s)
    ncA = _prog_cache[key]
    in_maps = [{"xc": to_pm(XT_c[k]), "xs": to_pm(XT_s[k])}
               for k in range(N_CORES)]
    resA = _run_spmd(ncA, in_maps)
    mom = np.zeros((2, NUM_LABELS, HALF, P, C), np.float64)
    for k in range(N_CORES):
        mom += resA[k]["mom"].astype(np.float64)
    S_c = mom[0].reshape(NUM_LABELS, C, C)
    S_s = mom[1].reshape(NUM_LABELS, C, C)

    # ---- host: means, covariances, Cholesky, transforms ----
    try:
        from scipy.linalg import solve_triangular

        def _tri_inv(L):
            return solve_triangular(L, np.eye(C), lower=True)
    except Exception:
        def _tri_inv(L):
            return np.linalg.solve(L, np.eye(C))

    Tm = np.zeros((len(glabels), C, C), np.float64)
    bias = np.zeros((len(glabels), C), np.float64)
    ok = [False] * len(glabels)
    for li, l in enumerate(glabels):
        a = counts_c[l]
        b = counts_s[l]
        sum_c = np.zeros(C, np.float64)
        sum_s = np.zeros(C, np.float64)
        for k in range(N_CORES):
            sum_c += XT_c[k, offs_c[li]:offs_c[li + 1]].sum(
                axis=0, dtype=np.float64)
            sum_s += XT_s[k, offs_s[li]:offs_s[li + 1]].sum(
                axis=0, dtype=np.float64)
        mu_c = sum_c / max(a, 1.0)
        mu_s = sum_s / max(b, 1.0)
        cov_c = (S_c[li] - a * np.outer(mu_c, mu_c)) / max(a - 1.0, 1.0)
        cov_s = (S_s[li] - b * np.outer(mu_s, mu_s)) / max(b - 1.0, 1.0)
        try:
            Lc = np.linalg.cholesky(cov_c)
            Ls = np.linalg.cholesky(cov_s)
            T = Ls @ _tri_inv(Lc)
        except np.linalg.LinAlgError:
            continue
        Tm[li] = T
        bias[li] = mu_s - T @ mu_c
        ok[li] = True

    if not any(ok):
        return out

    # ---- phase B on device: y = T_l x + c_l ----
    Xc = np.zeros((N_CORES, HALF * P, ppad_c), NP_B_IN)
    for k in range(N_CORES):
        Xc[k] = XT_c[k].T.astype(NP_B_IN, copy=False)

    tmat = np.zeros((len(glabels), HALF, HALF, P, P), NP_B_IN)
    for li in range(len(glabels)):
        Tl = Tm[li] if ok[li] else np.eye(C)
        for ci in range(HALF):
            for co in range(HALF):
                tmat[li, ci, co] = Tl[co * P:(co + 1) * P,
                                      ci * P:(ci + 1) * P].T
    bvec = np.zeros((HALF * P, len(glabels)), np.float32)
    for li in range(len(glabels)):
        if ok[li]:
            bvec[:, li] = bias[li]

    key = ("B", tuple(caps_c))
    if key not in _prog_cache:
        _prog_cache[key] = build_phase_b(caps_c)
    ncB = _prog_cache[key]
    in_maps = [{"x": Xc[k], "tmat": tmat, "bvec": bvec}
               for k in range(N_CORES)]
    resB = _run_spmd(ncB, in_maps)

    # ---- scatter back ----
    out2 = out.reshape(Cc, N)
    for k in range(N_CORES):
        Y = resB[k]["y"].astype(np.float32, copy=False)
        for li in range(len(glabels)):
            if not ok[li]:
                continue
            seg = segs_c[k][li]
            out2[:, seg] = Y[:, offs_c[li]:offs_c[li] + len(seg)]
    return out



# revision 2
# speedup vs baseline: 1.4010x; 1.4010x over previous
"""Class-wise whitening-coloring transform (CWCT) on 8 Trainium2 NeuronCores.

Strategy (pixels sorted by segmentation label on the host):
 * Phase A (device): per-label second moments S_l = sum x x^T.  One LABEL per
   CORE: core k receives every (content+style) pixel of guided label k as
   fp8e4m3 (prescaled by 1/4), laid out partition-major in 256-pixel groups.
   DoubleRow fp8 matmuls contract 256 pixels per instruction at 2x rate, and
   only the upper 128x256 strip plus the lower-right 128x128 block of the
   symmetric S are computed.  Moments return as fp16 [128, 384] per tensor —
   ~0.4 MB per core, so phase A is essentially pure input-DMA time.
 * Host: exact per-label means from the fp32 input (one [C,N]@[N,L] GEMM),
   covariances from the device moments, float64 Cholesky -> T_l and bias.
 * Phase B (device): y = T_l x + c_l over each label's pixel run, pixels
   split evenly across all 8 cores (channel-major fp16, PE matmuls, ACT
   bias+cast, output DMA on the otherwise-idle Pool/SWDGE queue).
 * Host scatters transformed pixels back into the full [1,256,512,512] map.

fp8 covariance inputs add ~3.3e-3 relative error end-to-end (measured on the
actual input distribution); fp16 phase B adds ~3e-4.  Budget is 2e-2.
"""
import os
import sys

for _p in ("/opt/trn_rl_repo", "/root/.axon_site/_ro/trn_rl_repo"):
    if os.path.isdir(_p) and _p not in sys.path:
        sys.path.insert(0, _p)

# The bass kernels execute through jax's axon platform; make sure it is
# available even if the calling process pinned JAX_PLATFORMS=cpu.
if "jax" not in sys.modules:
    _plat = os.environ.get("JAX_PLATFORMS", "")
    if _plat and "axon" not in _plat:
        os.environ["JAX_PLATFORMS"] = "axon," + _plat
    elif not _plat:
        os.environ["JAX_PLATFORMS"] = "axon,cpu"

import numpy as np
import ml_dtypes

import concourse.bass as bass
import concourse.tile as tile
from concourse import bacc, mybir

N_CORES = 8
NUM_LABELS = 8
C = 256
P = 128
HALF = 2  # channel halves (256 = 2*128)
GPX = 2 * P  # pixels per phase-A DoubleRow group

DT_A = mybir.dt.float8e4
NP_A = ml_dtypes.float8_e4m3
PRESCALE_A = 0.25  # keep fp16 moment outputs far from overflow
DT_B = mybir.dt.float16
NP_B = np.float16

CHUNK_A = 64     # phase A 256-px groups per DMA (32 KB/partition)
CHUNK_B = 2048   # phase B pixels per DMA chunk
MMW_B = 512      # phase B matmul moving width

_prog_cache = {}


def _new_nc():
    return bacc.Bacc("TRN2", target_bir_lowering=False, debug=False,
                     num_devices=N_CORES)


def build_phase_a(groups_c, groups_s, repeat=1):
    """One label per core.  groups_c/s: 256-pixel group counts (max over
    labels; smaller labels are zero-padded by the host).

    Input layout [128, (gc+gs)*512] fp8: free offset g*512 + j*256 + c holds
    pixel (g*256 + j*128 + partition), channel c; content groups first.
    Output mom[i] = [128, 384] fp16: S[0:128, 0:256] ++ S[128:256, 128:256],
    scaled by PRESCALE_A^2.
    """
    nc = _new_nc()
    xin = nc.dram_tensor("xin", [P, (groups_c + groups_s) * 2 * C], DT_A,
                         kind="ExternalInput")
    mom = nc.dram_tensor("mom", [2, P, C + P], mybir.dt.float16,
                         kind="ExternalOutput")
    DR = mybir.MatmulPerfMode.DoubleRow
    with tile.TileContext(nc) as tc:
        with (
            tc.tile_pool(name="in", bufs=3) as pin,
            tc.tile_pool(name="ps", bufs=2, space="PSUM") as pps,
            tc.tile_pool(name="so", bufs=2) as pout,
        ):
            def body_a(_=None):
                for i, (goff, ngrp) in enumerate(
                        [(0, groups_c), (groups_c, groups_s)]):
                    pst = pps.tile([P, C], mybir.dt.float32)
                    ps1 = pps.tile([P, P], mybir.dt.float32)
                    done = 0
                    while done < ngrp:
                        cur = min(ngrp - done, CHUNK_A)
                        xt = pin.tile([P, CHUNK_A * 2 * C], DT_A,
                                      tag="achunk")
                        nc.sync.dma_start(
                            xt[:, 0:cur * 2 * C],
                            xin[:, (goff + done) * 2 * C:
                                (goff + done + cur) * 2 * C])
                        for g in range(cur):
                            xv = xt[:, bass.ts(g, 2 * C)].rearrange(
                                "p (j c) -> p j c", j=2)
                            st = (done + g == 0)
                            sp = (done + g == ngrp - 1)
                            nc.tensor.matmul(pst[:], xv[:, :, 0:P], xv,
                                             start=st, stop=sp, perf_mode=DR)
                            nc.tensor.matmul(ps1[:], xv[:, :, P:C],
                                             xv[:, :, P:C],
                                             start=st, stop=sp, perf_mode=DR)
                        done += cur
                    so = pout.tile([P, C + P], mybir.dt.float16)
                    nc.vector.tensor_copy(so[:, 0:C], pst[:])
                    nc.vector.tensor_copy(so[:, C:], ps1[:])
                    nc.scalar.dma_start(mom[i], so[:])
            if repeat == 1:
                body_a()
            else:
                with tc.For_i(0, repeat, 1):
                    body_a()
    nc.compile()
    return nc


def build_phase_b(caps, repeat=1):
    """caps: per processed label, per-core pixel capacity (mult of 128)."""
    nL = len(caps)
    ppad = sum(caps)
    nc = _new_nc()
    x = nc.dram_tensor("x", [HALF * P, ppad], DT_B, kind="ExternalInput")
    tmat = nc.dram_tensor("tmat", [P, nL * HALF * HALF * P], DT_B,
                          kind="ExternalInput")
    bvec = nc.dram_tensor("bvec", [P, HALF * nL], mybir.dt.float32,
                          kind="ExternalInput")
    y = nc.dram_tensor("y", [HALF * P, ppad], DT_B, kind="ExternalOutput")
    xv = x.rearrange("(h p) n -> p h n", h=HALF)
    yv = y.rearrange("(h p) n -> p h n", h=HALF)
    with tile.TileContext(nc) as tc:
        with (
            tc.tile_pool(name="in", bufs=3) as pin,
            tc.tile_pool(name="tm", bufs=1) as ptm,
            tc.tile_pool(name="bias", bufs=1) as pb,
            tc.tile_pool(name="ps", bufs=4, space="PSUM") as pps,
            tc.tile_pool(name="out", bufs=3) as pout,
        ):
            tmsb = ptm.tile([P, nL * HALF * HALF * P], DT_B)
            nc.sync.dma_start(tmsb[:], tmat[:])
            tmv = tmsb[:].rearrange("p (l a b m) -> p l a b m",
                                    l=nL, a=HALF, b=HALF)
            bias = pb.tile([P, HALF * nL], mybir.dt.float32)
            nc.sync.dma_start(bias[:], bvec[:])

            def body_b(_=None):
                base = 0
                for li in range(nL):
                    cap = caps[li]
                    done = 0
                    while done < cap:
                        pxc = min(cap - done, CHUNK_B)
                        xt = pin.tile([P, HALF * CHUNK_B], DT_B,
                                      tag="bchunk")
                        nc.sync.dma_start(
                            xt[:, 0:HALF * pxc].rearrange(
                                "p (h n) -> p h n", h=HALF),
                            xv[:, :, base + done:base + done + pxc])
                        yt = pout.tile([P, HALF * CHUNK_B], DT_B,
                                       tag="bout")
                        m0 = 0
                        while m0 < pxc:
                            mw = min(pxc - m0, MMW_B)
                            for co in range(HALF):
                                ps = pps.tile([P, MMW_B], mybir.dt.float32)
                                for ci in range(HALF):
                                    nc.tensor.matmul(
                                        ps[:, 0:mw],
                                        tmv[:, li, ci, co, :],
                                        xt[:, ci * pxc + m0:
                                           ci * pxc + m0 + mw],
                                        start=(ci == 0), stop=(ci == 1))
                                nc.scalar.activation(
                                    yt[:, co * pxc + m0:co * pxc + m0 + mw],
                                    ps[:, 0:mw],
                                    mybir.ActivationFunctionType.Identity,
                                    bias=bias[:, co * nL + li:
                                              co * nL + li + 1])
                            m0 += mw
                        nc.gpsimd.dma_start(
                            yv[:, :, base + done:base + done + pxc],
                            yt[:, 0:HALF * pxc].rearrange(
                                "p (h n) -> p h n", h=HALF))
                        done += pxc
                    base += cap
            if repeat == 1:
                body_b()
            else:
                with tc.For_i(0, repeat, 1):
                    body_b()
    nc.compile()
    return nc


def _axon_devices():
    import jax
    try:
        devs = jax.devices("axon")
    except Exception:
        devs = jax.devices()
    assert len(devs) >= N_CORES, f"need {N_CORES} neuron cores, have {devs}"
    return devs[:N_CORES]


def _run_spmd(nc, in_maps):
    """SPMD execute `nc` on the 8 axon-tunneled NeuronCores."""
    import jax
    from jax.sharding import Mesh, PartitionSpec
    from jax.experimental.shard_map import shard_map
    from concourse.bass2jax import (_bass_exec_p, install_neuronx_cc_hook,
                                    partition_id_tensor)

    install_neuronx_cc_hook()
    partition_name = (nc.partition_id_tensor.name
                      if nc.partition_id_tensor else None)
    in_names, out_names, out_avals, zero_outs = [], [], [], []
    for alloc in nc.m.functions[0].allocations:
        if not isinstance(alloc, mybir.MemoryLocationSet):
            continue
        name = alloc.memorylocations[0].name
        if alloc.kind == "ExternalInput":
            if name != partition_name:
                in_names.append(name)
        elif alloc.kind == "ExternalOutput":
            shape = tuple(alloc.tensor_shape)
            dtype = mybir.dt.np(alloc.dtype)
            out_names.append(name)
            out_avals.append(jax.core.ShapedArray(shape, dtype))
            zero_outs.append(np.zeros(shape, dtype))
    n_params = len(in_names)
    all_in_names = list(in_names) + list(out_names)
    if partition_name is not None:
        all_in_names.append(partition_name)

    def _body(*args):
        operands = list(args)
        if partition_name is not None:
            operands.append(partition_id_tensor())
        outs = _bass_exec_p.bind(
            *operands,
            out_avals=tuple(out_avals),
            in_names=tuple(all_in_names),
            out_names=tuple(out_names),
            lowering_input_output_aliases=(),
            sim_require_finite=True,
            sim_require_nnan=True,
            nc=nc,
        )
        return tuple(outs)

    mesh = Mesh(np.asarray(_axon_devices()), ("core",))
    in_specs = (PartitionSpec("core"),) * (n_params + len(out_names))
    out_specs = (PartitionSpec("core"),) * len(out_names)
    fn = jax.jit(
        shard_map(_body, mesh=mesh, in_specs=in_specs, out_specs=out_specs,
                  check_rep=False),
        keep_unused=True,
    )
    concat_in = [
        np.concatenate([np.asarray(in_maps[c][n]) for c in range(N_CORES)], 0)
        for n in in_names
    ]
    concat_zero = [
        np.zeros((N_CORES * z.shape[0], *z.shape[1:]), z.dtype)
        for z in zero_outs
    ]
    outs = fn(*concat_in, *concat_zero)
    res = []
    for c in range(N_CORES):
        d = {}
        for i, name in enumerate(out_names):
            a = np.asarray(outs[i]).reshape(N_CORES, *out_avals[i].shape)
            d[name] = a[c]
        res.append(d)
    return res


def _split_sizes(count, parts):
    q, r = divmod(count, parts)
    return [q + (1 if k < r else 0) for k in range(parts)]


def _prepare(lab, guide_labels):
    """Sort pixel indices by label, split per core (for phase B).

    Returns: segs[k][li] = index array for core k, processed-label li;
             caps[li] = padded per-core capacity (multiple of 128).
    """
    order = np.argsort(lab, kind="stable")
    counts = np.bincount(lab, minlength=NUM_LABELS)
    starts = np.concatenate([[0], np.cumsum(counts)[:-1]])
    segs = [[] for _ in range(N_CORES)]
    caps = []
    for l in guide_labels:
        cnt = int(counts[l])
        sizes = _split_sizes(cnt, N_CORES)
        cap = max((max(sizes) + P - 1) // P * P, P)
        caps.append(cap)
        off = int(starts[l])
        for k in range(N_CORES):
            segs[k].append(order[off:off + sizes[k]])
            off += sizes[k]
    return segs, caps, counts


def _to_groups(a8, ngroups):
    """[npx<=ngroups*256, C] fp8 -> [128, ngroups*512] partition-major."""
    out = np.zeros((ngroups * 2 * P, C), NP_A)
    out[:a8.shape[0]] = a8
    t = out.reshape(ngroups, 2, P, C).transpose(2, 0, 1, 3)
    return np.ascontiguousarray(t).reshape(P, -1)


def kernel(content_feat, style_feat, content_seg, style_seg):
    content_feat = np.asarray(content_feat)
    style_feat = np.asarray(style_feat)
    content_seg = np.asarray(content_seg)
    style_seg = np.asarray(style_seg)

    B, Cc, H, W = content_feat.shape
    N = H * W
    x = content_feat.reshape(Cc, N)
    s = style_feat.reshape(Cc, N)
    labc = content_seg.reshape(-1)
    labs = style_seg.reshape(-1)

    counts_c = np.bincount(labc, minlength=NUM_LABELS).astype(np.float64)
    counts_s = np.bincount(labs, minlength=NUM_LABELS).astype(np.float64)
    guide = [(counts_c[l] > 10) and (counts_s[l] > 10)
             and (counts_c[l] < 100.0 * counts_s[l])
             and (counts_s[l] < 100.0 * counts_c[l])
             for l in range(NUM_LABELS)]
    glabels = [l for l in range(NUM_LABELS) if guide[l]]
    out = content_feat.astype(np.float32, copy=True)
    if not glabels:
        return out

    nL = len(glabels)
    order_c = np.argsort(labc, kind="stable")
    order_s = np.argsort(labs, kind="stable")
    starts_c = np.concatenate(
        [[0], np.cumsum(np.bincount(labc, minlength=NUM_LABELS))])
    starts_s = np.concatenate(
        [[0], np.cumsum(np.bincount(labs, minlength=NUM_LABELS))])

    # fp8 prescaled pixel-major copies for phase A gathers
    x8 = (x.T * PRESCALE_A).astype(NP_A)   # [N, C]
    s8 = (s.T * PRESCALE_A).astype(NP_A)

    GC = max(max((int(counts_c[l]) + GPX - 1) // GPX for l in glabels), 1)
    GS = max(max((int(counts_s[l]) + GPX - 1) // GPX for l in glabels), 1)

    in_maps = []
    for k in range(N_CORES):
        if k < nL:
            l = glabels[k]
            pc = x8[order_c[starts_c[l]:starts_c[l + 1]]]
            ps = s8[order_s[starts_s[l]:starts_s[l + 1]]]
            xin = np.concatenate([_to_groups(pc, GC), _to_groups(ps, GS)], 1)
        else:
            xin = np.zeros((P, (GC + GS) * 2 * C), NP_A)
        in_maps.append({"xin": xin})

    key = ("A", GC, GS)
    if key not in _prog_cache:
        _prog_cache[key] = build_phase_a(GC, GS)
    resA = _run_spmd(_prog_cache[key], in_maps)

    inv_sc2 = 1.0 / (PRESCALE_A * PRESCALE_A)
    S_c = np.zeros((nL, C, C), np.float64)
    S_s = np.zeros((nL, C, C), np.float64)
    for li in range(nL):
        m = resA[li]["mom"].astype(np.float64) * inv_sc2
        for Sd, mi in ((S_c, m[0]), (S_s, m[1])):
            Sd[li, 0:P, :] = mi[:, 0:C]
            Sd[li, P:, P:] = mi[:, C:]
            Sd[li, P:, 0:P] = mi[:, P:C].T

    # exact per-label sums via one GEMM per tensor
    onehot_c = np.zeros((N, NUM_LABELS))
    onehot_c[np.arange(N), labc] = 1.0
    onehot_s = np.zeros((N, NUM_LABELS))
    onehot_s[np.arange(N), labs] = 1.0
    sums_c = x.astype(np.float64) @ onehot_c  # [C, NUM_LABELS]
    sums_s = s.astype(np.float64) @ onehot_s

    try:
        from scipy.linalg import solve_triangular

        def _tri_inv(L):
            return solve_triangular(L, np.eye(C), lower=True)
    except Exception:
        def _tri_inv(L):
            return np.linalg.solve(L, np.eye(C))

    Tm = np.zeros((nL, C, C), np.float64)
    bias = np.zeros((nL, C), np.float64)
    ok = [False] * nL
    for li, l in enumerate(glabels):
        a = counts_c[l]
        b = counts_s[l]
        mu_c = sums_c[:, l] / max(a, 1.0)
        mu_s = sums_s[:, l] / max(b, 1.0)
        cov_c = (S_c[li] - a * np.outer(mu_c, mu_c)) / max(a - 1.0, 1.0)
        cov_s = (S_s[li] - b * np.outer(mu_s, mu_s)) / max(b - 1.0, 1.0)
        try:
            Lc = np.linalg.cholesky(cov_c)
            Ls = np.linalg.cholesky(cov_s)
            T = Ls @ _tri_inv(Lc)
        except np.linalg.LinAlgError:
            continue
        Tm[li] = T
        bias[li] = mu_s - T @ mu_c
        ok[li] = True

    if not any(ok):
        return out

    # ---- phase B on device: y = T_l x + c_l ----
    segs_c, caps_c, _ = _prepare(labc, glabels)
    ppad_c = sum(caps_c)
    offs_c = np.concatenate([[0], np.cumsum(caps_c)]).astype(int)

    xt16 = x.T.astype(NP_B)  # [N, C] pixel-major fp16
    Xc = np.zeros((N_CORES, HALF * P, ppad_c), NP_B)
    for k in range(N_CORES):
        col = np.zeros((ppad_c, C), NP_B)
        for li in range(nL):
            seg = segs_c[k][li]
            col[offs_c[li]:offs_c[li] + len(seg)] = xt16[seg]
        Xc[k] = col.T

    tmat = np.zeros((P, nL, HALF, HALF, P), NP_B)
    for li in range(nL):
        Tl = Tm[li] if ok[li] else np.eye(C)
        for ci in range(HALF):
            for co in range(HALF):
                tmat[:, li, ci, co, :] = Tl[co * P:(co + 1) * P,
                                            ci * P:(ci + 1) * P].T
    tmat = tmat.reshape(P, nL * HALF * HALF * P)
    bvec = np.zeros((P, HALF * nL), np.float32)
    for li in range(nL):
        if ok[li]:
            bvec[:, 0 * nL + li] = bias[li][0:P]
            bvec[:, 1 * nL + li] = bias[li][P:C]

    key = ("B", tuple(caps_c))
    if key not in _prog_cache:
        _prog_cache[key] = build_phase_b(caps_c)
    in_maps = [{"x": Xc[k], "tmat": tmat, "bvec": bvec}
               for k in range(N_CORES)]
    resB = _run_spmd(_prog_cache[key], in_maps)

    # ---- scatter back ----
    out2 = out.reshape(Cc, N)
    for k in range(N_CORES):
        Y = resB[k]["y"].astype(np.float32, copy=False)
        for li in range(nL):
            if not ok[li]:
                continue
            seg = segs_c[k][li]
            out2[:, seg] = Y[:, offs_c[li]:offs_c[li] + len(seg)]
    return out


# revision 3
# speedup vs baseline: 1.6114x; 1.1502x over previous
"""Class-wise whitening-coloring transform (CWCT) on 8 Trainium2 NeuronCores.

Strategy (pixels sorted by segmentation label on the host):
 * Phase A (device): per-label second moments S_l = sum x x^T.  One LABEL per
   CORE: core k receives every (content+style) pixel of guided label k as
   fp8e4m3 (prescaled by 1/4), laid out partition-major in 256-pixel groups.
   DoubleRow fp8 matmuls contract 256 pixels per instruction at 2x rate, and
   only the upper 128x256 strip plus the lower-right 128x128 block of the
   symmetric S are computed.  Moments return as fp16 [128, 384] per tensor —
   ~0.4 MB per core, so phase A is essentially pure input-DMA time.
 * Host: exact per-label means from the fp32 input (one [C,N]@[N,L] GEMM),
   covariances from the device moments, float64 Cholesky -> T_l and bias.
 * Phase B (device): y = T_l x + c_l over each label's pixel run, pixels
   split evenly across all 8 cores (channel-major fp16, PE matmuls, ACT
   bias+cast, output DMA on the otherwise-idle Pool/SWDGE queue).
 * Host scatters transformed pixels back into the full [1,256,512,512] map.

fp8 covariance inputs add ~3.3e-3 relative error end-to-end (measured on the
actual input distribution); fp16 phase B adds ~3e-4.  Budget is 2e-2.
"""
import os
import sys

for _p in ("/opt/trn_rl_repo", "/root/.axon_site/_ro/trn_rl_repo"):
    if os.path.isdir(_p) and _p not in sys.path:
        sys.path.insert(0, _p)

# The bass kernels execute through jax's axon platform; make sure it is
# available even if the calling process pinned JAX_PLATFORMS=cpu.
if "jax" not in sys.modules:
    _plat = os.environ.get("JAX_PLATFORMS", "")
    if _plat and "axon" not in _plat:
        os.environ["JAX_PLATFORMS"] = "axon," + _plat
    elif not _plat:
        os.environ["JAX_PLATFORMS"] = "axon,cpu"

import numpy as np
import ml_dtypes

import concourse.bass as bass
import concourse.tile as tile
from concourse import bacc, mybir

N_CORES = 8
NUM_LABELS = 8
C = 256
P = 128
HALF = 2  # channel halves (256 = 2*128)
GPX = 2 * P  # pixels per phase-A DoubleRow group

DT_A = mybir.dt.float8e4
NP_A = ml_dtypes.float8_e4m3
PRESCALE_A = 0.25  # keep fp16 moment outputs far from overflow
DT_B = mybir.dt.float16
NP_B = np.float16

CHUNK_A = 6      # phase A 256-px groups per DMA (3 KB/partition)
CHUNK_B = 2048   # phase B pixels per DMA chunk
MMW_B = 512      # phase B matmul moving width

_prog_cache = {}


def _new_nc():
    return bacc.Bacc("TRN2", target_bir_lowering=False, debug=False,
                     num_devices=N_CORES)


def build_phase_a(groups_c, groups_s, repeat=1):
    """One label per core.  groups_c/s: 256-pixel group counts (max over
    labels; smaller labels are zero-padded by the host).

    Input layout [128, (gc+gs)*512] fp8: free offset g*512 + j*256 + c holds
    pixel (g*256 + j*128 + partition), channel c; content groups first.
    Output mom[i] = [128, 384] fp16: S[0:128, 0:256] ++ S[128:256, 128:256],
    scaled by PRESCALE_A^2.
    """
    nc = _new_nc()
    xin = nc.dram_tensor("xin", [P, (groups_c + groups_s) * 2 * C], DT_A,
                         kind="ExternalInput")
    mom = nc.dram_tensor("mom", [2, P, C + P], mybir.dt.float16,
                         kind="ExternalOutput")
    DR = mybir.MatmulPerfMode.DoubleRow
    with tile.TileContext(nc) as tc:
        with (
            tc.tile_pool(name="in", bufs=12) as pin,
            tc.tile_pool(name="ps", bufs=2, space="PSUM") as pps,
            tc.tile_pool(name="so", bufs=2) as pout,
        ):
            def body_a(_=None):
                for i, (goff, ngrp) in enumerate(
                        [(0, groups_c), (groups_c, groups_s)]):
                    pst = pps.tile([P, C], mybir.dt.float32)
                    ps1 = pps.tile([P, P], mybir.dt.float32)
                    done = 0
                    while done < ngrp:
                        cur = min(ngrp - done, CHUNK_A)
                        xt = pin.tile([P, CHUNK_A * 2 * C], DT_A,
                                      tag="achunk")
                        nc.sync.dma_start(
                            xt[:, 0:cur * 2 * C],
                            xin[:, (goff + done) * 2 * C:
                                (goff + done + cur) * 2 * C])
                        for g in range(cur):
                            xv = xt[:, bass.ts(g, 2 * C)].rearrange(
                                "p (j c) -> p j c", j=2)
                            st = (done + g == 0)
                            sp = (done + g == ngrp - 1)
                            nc.tensor.matmul(pst[:], xv[:, :, 0:P], xv,
                                             start=st, stop=sp, perf_mode=DR)
                            nc.tensor.matmul(ps1[:], xv[:, :, P:C],
                                             xv[:, :, P:C],
                                             start=st, stop=sp, perf_mode=DR)
                        done += cur
                    so = pout.tile([P, C + P], mybir.dt.float16)
                    nc.vector.tensor_copy(so[:, 0:C], pst[:])
                    nc.vector.tensor_copy(so[:, C:], ps1[:])
                    nc.scalar.dma_start(mom[i], so[:])
            if repeat == 1:
                body_a()
            else:
                with tc.For_i(0, repeat, 1):
                    body_a()
    nc.compile()
    return nc


def build_phase_b(caps, repeat=1):
    """caps: per processed label, per-core pixel capacity (mult of 128)."""
    nL = len(caps)
    ppad = sum(caps)
    nc = _new_nc()
    x = nc.dram_tensor("x", [HALF * P, ppad], DT_B, kind="ExternalInput")
    tmat = nc.dram_tensor("tmat", [P, nL * HALF * HALF * P], DT_B,
                          kind="ExternalInput")
    bvec = nc.dram_tensor("bvec", [P, HALF * nL], mybir.dt.float32,
                          kind="ExternalInput")
    y = nc.dram_tensor("y", [HALF * P, ppad], DT_B, kind="ExternalOutput")
    xv = x.rearrange("(h p) n -> p h n", h=HALF)
    yv = y.rearrange("(h p) n -> p h n", h=HALF)
    with tile.TileContext(nc) as tc:
        with (
            tc.tile_pool(name="in", bufs=6) as pin,
            tc.tile_pool(name="tm", bufs=1) as ptm,
            tc.tile_pool(name="bias", bufs=1) as pb,
            tc.tile_pool(name="ps", bufs=4, space="PSUM") as pps,
            tc.tile_pool(name="out", bufs=6) as pout,
        ):
            tmsb = ptm.tile([P, nL * HALF * HALF * P], DT_B)
            nc.sync.dma_start(tmsb[:], tmat[:])
            tmv = tmsb[:].rearrange("p (l a b m) -> p l a b m",
                                    l=nL, a=HALF, b=HALF)
            bias = pb.tile([P, HALF * nL], mybir.dt.float32)
            nc.sync.dma_start(bias[:], bvec[:])

            def body_b(_=None):
                base = 0
                for li in range(nL):
                    cap = caps[li]
                    done = 0
                    while done < cap:
                        pxc = min(cap - done, CHUNK_B)
                        xt = pin.tile([P, HALF * CHUNK_B], DT_B,
                                      tag="bchunk")
                        nc.sync.dma_start(
                            xt[:, 0:HALF * pxc].rearrange(
                                "p (h n) -> p h n", h=HALF),
                            xv[:, :, base + done:base + done + pxc])
                        yt = pout.tile([P, HALF * CHUNK_B], DT_B,
                                       tag="bout")
                        m0 = 0
                        while m0 < pxc:
                            mw = min(pxc - m0, MMW_B)
                            for co in range(HALF):
                                ps = pps.tile([P, MMW_B], mybir.dt.float32)
                                for ci in range(HALF):
                                    nc.tensor.matmul(
                                        ps[:, 0:mw],
                                        tmv[:, li, ci, co, :],
                                        xt[:, ci * pxc + m0:
                                           ci * pxc + m0 + mw],
                                        start=(ci == 0), stop=(ci == 1))
                                nc.scalar.activation(
                                    yt[:, co * pxc + m0:co * pxc + m0 + mw],
                                    ps[:, 0:mw],
                                    mybir.ActivationFunctionType.Identity,
                                    bias=bias[:, co * nL + li:
                                              co * nL + li + 1])
                            m0 += mw
                        nc.gpsimd.dma_start(
                            yv[:, :, base + done:base + done + pxc],
                            yt[:, 0:HALF * pxc].rearrange(
                                "p (h n) -> p h n", h=HALF))
                        done += pxc
                    base += cap
            if repeat == 1:
                body_b()
            else:
                with tc.For_i(0, repeat, 1):
                    body_b()
    nc.compile()
    return nc


def _axon_devices():
    import jax
    try:
        devs = jax.devices("axon")
    except Exception:
        devs = jax.devices()
    assert len(devs) >= N_CORES, f"need {N_CORES} neuron cores, have {devs}"
    return devs[:N_CORES]


def _run_spmd(nc, in_maps):
    """SPMD execute `nc` on the 8 axon-tunneled NeuronCores."""
    import jax
    from jax.sharding import Mesh, PartitionSpec
    from jax.experimental.shard_map import shard_map
    from concourse.bass2jax import (_bass_exec_p, install_neuronx_cc_hook,
                                    partition_id_tensor)

    install_neuronx_cc_hook()
    partition_name = (nc.partition_id_tensor.name
                      if nc.partition_id_tensor else None)
    in_names, out_names, out_avals, zero_outs = [], [], [], []
    for alloc in nc.m.functions[0].allocations:
        if not isinstance(alloc, mybir.MemoryLocationSet):
            continue
        name = alloc.memorylocations[0].name
        if alloc.kind == "ExternalInput":
            if name != partition_name:
                in_names.append(name)
        elif alloc.kind == "ExternalOutput":
            shape = tuple(alloc.tensor_shape)
            dtype = mybir.dt.np(alloc.dtype)
            out_names.append(name)
            out_avals.append(jax.core.ShapedArray(shape, dtype))
            zero_outs.append(np.zeros(shape, dtype))
    n_params = len(in_names)
    all_in_names = list(in_names) + list(out_names)
    if partition_name is not None:
        all_in_names.append(partition_name)

    def _body(*args):
        operands = list(args)
        if partition_name is not None:
            operands.append(partition_id_tensor())
        outs = _bass_exec_p.bind(
            *operands,
            out_avals=tuple(out_avals),
            in_names=tuple(all_in_names),
            out_names=tuple(out_names),
            lowering_input_output_aliases=(),
            sim_require_finite=True,
            sim_require_nnan=True,
            nc=nc,
        )
        return tuple(outs)

    mesh = Mesh(np.asarray(_axon_devices()), ("core",))
    in_specs = (PartitionSpec("core"),) * (n_params + len(out_names))
    out_specs = (PartitionSpec("core"),) * len(out_names)
    fn = jax.jit(
        shard_map(_body, mesh=mesh, in_specs=in_specs, out_specs=out_specs,
                  check_rep=False),
        keep_unused=True,
    )
    concat_in = [
        np.concatenate([np.asarray(in_maps[c][n]) for c in range(N_CORES)], 0)
        for n in in_names
    ]
    concat_zero = [
        np.zeros((N_CORES * z.shape[0], *z.shape[1:]), z.dtype)
        for z in zero_outs
    ]
    outs = fn(*concat_in, *concat_zero)
    res = []
    for c in range(N_CORES):
        d = {}
        for i, name in enumerate(out_names):
            a = np.asarray(outs[i]).reshape(N_CORES, *out_avals[i].shape)
            d[name] = a[c]
        res.append(d)
    return res


def _split_sizes(count, parts):
    q, r = divmod(count, parts)
    return [q + (1 if k < r else 0) for k in range(parts)]


def _prepare(lab, guide_labels):
    """Sort pixel indices by label, split per core (for phase B).

    Returns: segs[k][li] = index array for core k, processed-label li;
             caps[li] = padded per-core capacity (multiple of 128).
    """
    order = np.argsort(lab, kind="stable")
    counts = np.bincount(lab, minlength=NUM_LABELS)
    starts = np.concatenate([[0], np.cumsum(counts)[:-1]])
    segs = [[] for _ in range(N_CORES)]
    caps = []
    for l in guide_labels:
        cnt = int(counts[l])
        sizes = _split_sizes(cnt, N_CORES)
        cap = max((max(sizes) + P - 1) // P * P, P)
        caps.append(cap)
        off = int(starts[l])
        for k in range(N_CORES):
            segs[k].append(order[off:off + sizes[k]])
            off += sizes[k]
    return segs, caps, counts


def _to_groups(a8, ngroups):
    """[npx<=ngroups*256, C] fp8 -> [128, ngroups*512] partition-major."""
    out = np.zeros((ngroups * 2 * P, C), NP_A)
    out[:a8.shape[0]] = a8
    t = out.reshape(ngroups, 2, P, C).transpose(2, 0, 1, 3)
    return np.ascontiguousarray(t).reshape(P, -1)


def kernel(content_feat, style_feat, content_seg, style_seg):
    content_feat = np.asarray(content_feat)
    style_feat = np.asarray(style_feat)
    content_seg = np.asarray(content_seg)
    style_seg = np.asarray(style_seg)

    B, Cc, H, W = content_feat.shape
    N = H * W
    x = content_feat.reshape(Cc, N)
    s = style_feat.reshape(Cc, N)
    labc = content_seg.reshape(-1)
    labs = style_seg.reshape(-1)

    counts_c = np.bincount(labc, minlength=NUM_LABELS).astype(np.float64)
    counts_s = np.bincount(labs, minlength=NUM_LABELS).astype(np.float64)
    guide = [(counts_c[l] > 10) and (counts_s[l] > 10)
             and (counts_c[l] < 100.0 * counts_s[l])
             and (counts_s[l] < 100.0 * counts_c[l])
             for l in range(NUM_LABELS)]
    glabels = [l for l in range(NUM_LABELS) if guide[l]]
    out = content_feat.astype(np.float32, copy=True)
    if not glabels:
        return out

    nL = len(glabels)
    order_c = np.argsort(labc, kind="stable")
    order_s = np.argsort(labs, kind="stable")
    starts_c = np.concatenate(
        [[0], np.cumsum(np.bincount(labc, minlength=NUM_LABELS))])
    starts_s = np.concatenate(
        [[0], np.cumsum(np.bincount(labs, minlength=NUM_LABELS))])

    # fp8 prescaled pixel-major copies for phase A gathers
    x8 = (x.T * PRESCALE_A).astype(NP_A)   # [N, C]
    s8 = (s.T * PRESCALE_A).astype(NP_A)

    GC = max(max((int(counts_c[l]) + GPX - 1) // GPX for l in glabels), 1)
    GS = max(max((int(counts_s[l]) + GPX - 1) // GPX for l in glabels), 1)

    in_maps = []
    for k in range(N_CORES):
        if k < nL:
            l = glabels[k]
            pc = x8[order_c[starts_c[l]:starts_c[l + 1]]]
            ps = s8[order_s[starts_s[l]:starts_s[l + 1]]]
            xin = np.concatenate([_to_groups(pc, GC), _to_groups(ps, GS)], 1)
        else:
            xin = np.zeros((P, (GC + GS) * 2 * C), NP_A)
        in_maps.append({"xin": xin})

    key = ("A", GC, GS)
    if key not in _prog_cache:
        _prog_cache[key] = build_phase_a(GC, GS)
    resA = _run_spmd(_prog_cache[key], in_maps)

    inv_sc2 = 1.0 / (PRESCALE_A * PRESCALE_A)
    S_c = np.zeros((nL, C, C), np.float64)
    S_s = np.zeros((nL, C, C), np.float64)
    for li in range(nL):
        m = resA[li]["mom"].astype(np.float64) * inv_sc2
        for Sd, mi in ((S_c, m[0]), (S_s, m[1])):
            Sd[li, 0:P, :] = mi[:, 0:C]
            Sd[li, P:, P:] = mi[:, C:]
            Sd[li, P:, 0:P] = mi[:, P:C].T

    # exact per-label sums via one GEMM per tensor
    onehot_c = np.zeros((N, NUM_LABELS))
    onehot_c[np.arange(N), labc] = 1.0
    onehot_s = np.zeros((N, NUM_LABELS))
    onehot_s[np.arange(N), labs] = 1.0
    sums_c = x.astype(np.float64) @ onehot_c  # [C, NUM_LABELS]
    sums_s = s.astype(np.float64) @ onehot_s

    try:
        from scipy.linalg import solve_triangular

        def _tri_inv(L):
            return solve_triangular(L, np.eye(C), lower=True)
    except Exception:
        def _tri_inv(L):
            return np.linalg.solve(L, np.eye(C))

    Tm = np.zeros((nL, C, C), np.float64)
    bias = np.zeros((nL, C), np.float64)
    ok = [False] * nL
    for li, l in enumerate(glabels):
        a = counts_c[l]
        b = counts_s[l]
        mu_c = sums_c[:, l] / max(a, 1.0)
        mu_s = sums_s[:, l] / max(b, 1.0)
        cov_c = (S_c[li] - a * np.outer(mu_c, mu_c)) / max(a - 1.0, 1.0)
        cov_s = (S_s[li] - b * np.outer(mu_s, mu_s)) / max(b - 1.0, 1.0)
        try:
            Lc = np.linalg.cholesky(cov_c)
            Ls = np.linalg.cholesky(cov_s)
            T = Ls @ _tri_inv(Lc)
        except np.linalg.LinAlgError:
            continue
        Tm[li] = T
        bias[li] = mu_s - T @ mu_c
        ok[li] = True

    if not any(ok):
        return out

    # ---- phase B on device: y = T_l x + c_l ----
    segs_c, caps_c, _ = _prepare(labc, glabels)
    ppad_c = sum(caps_c)
    offs_c = np.concatenate([[0], np.cumsum(caps_c)]).astype(int)

    xt16 = x.T.astype(NP_B)  # [N, C] pixel-major fp16
    Xc = np.zeros((N_CORES, HALF * P, ppad_c), NP_B)
    for k in range(N_CORES):
        col = np.zeros((ppad_c, C), NP_B)
        for li in range(nL):
            seg = segs_c[k][li]
            col[offs_c[li]:offs_c[li] + len(seg)] = xt16[seg]
        Xc[k] = col.T

    tmat = np.zeros((P, nL, HALF, HALF, P), NP_B)
    for li in range(nL):
        Tl = Tm[li] if ok[li] else np.eye(C)
        for ci in range(HALF):
            for co in range(HALF):
                tmat[:, li, ci, co, :] = Tl[co * P:(co + 1) * P,
                                            ci * P:(ci + 1) * P].T
    tmat = tmat.reshape(P, nL * HALF * HALF * P)
    bvec = np.zeros((P, HALF * nL), np.float32)
    for li in range(nL):
        if ok[li]:
            bvec[:, 0 * nL + li] = bias[li][0:P]
            bvec[:, 1 * nL + li] = bias[li][P:C]

    key = ("B", tuple(caps_c))
    if key not in _prog_cache:
        _prog_cache[key] = build_phase_b(caps_c)
    in_maps = [{"x": Xc[k], "tmat": tmat, "bvec": bvec}
               for k in range(N_CORES)]
    resB = _run_spmd(_prog_cache[key], in_maps)

    # ---- scatter back ----
    out2 = out.reshape(Cc, N)
    for k in range(N_CORES):
        Y = resB[k]["y"].astype(np.float32, copy=False)
        for li in range(nL):
            if not ok[li]:
                continue
            seg = segs_c[k][li]
            out2[:, seg] = Y[:, offs_c[li]:offs_c[li] + len(seg)]
    return out
